# revision 1
# baseline (speedup 1.0000x reference)
"""Trainium2 Bass kernel for nn_Entangle_layer_4054449127786.

The reference collapses to a per-branch 2x2 gate applied identically to every
qubit slice q (all 8 qubit indices are distinct, so the tiling loop in the
reference never re-tiles).  With c1 = 2^-0.25, c2 = 2^-0.75, inputs
s_re/s_im of shape (B=128, Q=8, R=64, 2, K=64):

  out[:, :, :64]  (X branch):  out[..., a, :] = c1 * s[..., 1-a, :]
  out[:, :, 64:]  (Z branch):  out[..., 0, :] = c2*(sr - si) + i*c2*(sr + si)
                               out[..., 1, :] = -(that, with a=1 inputs)

Pure data parallelism: batch dim 0 is sharded 16-per-core across 8 cores.
Per core: partition dim = (n, q) = 16*8 = 128, free dim = (r, a, k).

Device-side scheme per r-chunk (validated vs reference, rel err ~5e-8):
  1. ACT: in-place sign-scale u[a0] = c2*s[a0], u[a1] = -c2*s[a1]
  2. GPSIMD: X outputs x[a0] = -sqrt2*u[a1], x[a1] = sqrt2*u[a0]
  3. DVE: Z outputs zre = u_re - u_im, zim = u_re + u_im  (both a halves)
All DMAs via HWDGE (nc.sync), contiguous >=8KB runs per partition.
"""

import numpy as np

B = 128          # full batch
NCORES = 8
BC = B // NCORES  # batch per core = 16
Q = 8
RIN = 64
K = 64
P = BC * Q               # 128 partitions
F_IN = RIN * 2 * K       # 8192 f32 per partition in
F_OUT = 2 * F_IN         # 16384 f32 per partition out
CR = 16                  # r-values per chunk
C = CR * 2 * K           # 2048 f32 per partition per chunk

C2 = float(np.float32(2.0) ** np.float32(-0.75))
S2 = float(np.float32(2.0) ** np.float32(0.5))

# Set by test harnesses to capture a profile; ignored in normal use.
TRACE = False
LAST_RESULTS = None

_CACHE = {}


def _build():
    import concourse.bacc as bacc
    import concourse.tile as tile
    from concourse import mybir

    nc = bacc.Bacc("TRN2", target_bir_lowering=False, debug=False,
                   num_devices=NCORES)
    sr_d = nc.dram_tensor("s_re", [P, F_IN], mybir.dt.float32,
                          kind="ExternalInput").ap()
    si_d = nc.dram_tensor("s_im", [P, F_IN], mybir.dt.float32,
                          kind="ExternalInput").ap()
    or_d = nc.dram_tensor("o_re", [P, F_OUT], mybir.dt.float32,
                          kind="ExternalOutput").ap()
    oi_d = nc.dram_tensor("o_im", [P, F_OUT], mybir.dt.float32,
                          kind="ExternalOutput").ap()

    # (p, b, f): b=0 -> X half, b=1 -> Z half of each partition's output row
    or3 = or_d.rearrange("p (b f) -> p b f", b=2)
    oi3 = oi_d.rearrange("p (b f) -> p b f", b=2)

    with tile.TileContext(nc) as tc:
        with tc.tile_pool(name="inp", bufs=4) as inp, \
             tc.tile_pool(name="outp", bufs=3) as outp:
            for ci in range(RIN // CR):
                f0 = ci * C
                s_re = inp.tile([P, C], mybir.dt.float32, tag="s_re")
                s_im = inp.tile([P, C], mybir.dt.float32, tag="s_im")
                nc.sync.dma_start(out=s_re[:], in_=sr_d[:, f0:f0 + C])
                nc.sync.dma_start(out=s_im[:], in_=si_d[:, f0:f0 + C])

                o_re = outp.tile([P, 2 * C], mybir.dt.float32, tag="o_re")
                o_im = outp.tile([P, 2 * C], mybir.dt.float32, tag="o_im")

                sr4 = s_re[:].rearrange("p (r a k) -> p r a k", a=2, k=K)
                si4 = s_im[:].rearrange("p (r a k) -> p r a k", a=2, k=K)
                # X halves of the out tiles
                xr4 = o_re[:, 0:C].rearrange("p (r a k) -> p r a k", a=2, k=K)
                xi4 = o_im[:, 0:C].rearrange("p (r a k) -> p r a k", a=2, k=K)

                # 1. sign-scale in place on ACT: u[a0]=c2*s, u[a1]=-c2*s
                nc.scalar.mul(sr4[:, :, 0, :], sr4[:, :, 0, :], C2)
                nc.scalar.mul(sr4[:, :, 1, :], sr4[:, :, 1, :], -C2)
                nc.scalar.mul(si4[:, :, 0, :], si4[:, :, 0, :], C2)
                nc.scalar.mul(si4[:, :, 1, :], si4[:, :, 1, :], -C2)

                # 2. X branch on GPSIMD: swap a, scale by -/+sqrt2
                nc.gpsimd.tensor_scalar_mul(xr4[:, :, 0, :], sr4[:, :, 1, :], -S2)
                nc.gpsimd.tensor_scalar_mul(xr4[:, :, 1, :], sr4[:, :, 0, :], S2)
                nc.gpsimd.tensor_scalar_mul(xi4[:, :, 0, :], si4[:, :, 1, :], -S2)
                nc.gpsimd.tensor_scalar_mul(xi4[:, :, 1, :], si4[:, :, 0, :], S2)

                # 3. Z branch on DVE over both a halves at once
                nc.vector.tensor_sub(o_re[:, C:2 * C], s_re[:], s_im[:])
                nc.vector.tensor_add(o_im[:, C:2 * C], s_re[:], s_im[:])

                nc.sync.dma_start(
                    out=or3[:, :, f0:f0 + C],
                    in_=o_re[:].rearrange("p (b f) -> p b f", b=2))
                nc.sync.dma_start(
                    out=oi3[:, :, f0:f0 + C],
                    in_=o_im[:].rearrange("p (b f) -> p b f", b=2))

    nc.compile()
    return nc


def kernel(state_re: np.ndarray, state_im: np.ndarray) -> np.ndarray:
    global LAST_RESULTS
    from concourse.bass_utils import run_bass_kernel_spmd

    if "nc" not in _CACHE:
        _CACHE["nc"] = _build()
    nc = _CACHE["nc"]

    state_re = np.ascontiguousarray(state_re, dtype=np.float32)
    state_im = np.ascontiguousarray(state_im, dtype=np.float32)

    in_maps = []
    for c in range(NCORES):
        sl = slice(c * BC, (c + 1) * BC)
        in_maps.append({
            "s_re": state_re[sl].reshape(P, F_IN),
            "s_im": state_im[sl].reshape(P, F_IN),
        })

    res = run_bass_kernel_spmd(nc, in_maps, list(range(NCORES)), trace=TRACE)
    LAST_RESULTS = res

    out = np.empty((B, Q, 2 * RIN, 2, K), np.complex64)
    view = out.view(np.float32).reshape(B, Q, 2 * RIN, 2, K, 2)
    for c in range(NCORES):
        sl = slice(c * BC, (c + 1) * BC)
        view[sl, ..., 0] = res.results[c]["o_re"].reshape(BC, Q, 2 * RIN, 2, K)
        view[sl, ..., 1] = res.results[c]["o_im"].reshape(BC, Q, 2 * RIN, 2, K)
    return out
